# revision 3
# baseline (speedup 1.0000x reference)
"""Non-local block (B=8, C=256, H=W=56) as a Bass/Tile kernel on 8 trn2 NeuronCores.

Sharding: pure data parallelism - core i computes sample i end-to-end
(attention is per-sample, weights replicated). kernel() takes the full
inputs, builds per-core input maps, runs the SPMD Bass program, and
stacks the per-core outputs.

Per-core math (xf = x[i] reshaped [C, N], N = H*W = 3136, CH = 128):
  theta = w_theta @ xf          [CH, N]   (bf16)
  phi   = w_phi   @ xf          [CH, N]   (bf16)
  gT    = (w_g @ xf)^T          [N, CH]   (fp8 e4m3, m-major)
  S_T[m, n] = (phi^T theta)[m, n]; E = exp(S_T - 3.5)      (fp8 e4m3)
  y[c, n] = sum_m gT[m, c] E[m, n]   (fp8 DoubleRow, PSUM-accumulated)
  d[n]    = sum_m E[m, n]            (ones-matmul, fp8 DoubleRow)
  out = w_z @ (y / d) + xf      [C, N]
The exp bias (-3.5) keeps E inside fp8 range and cancels exactly in y/d.

Structure vs the previous version (121 us): seven uniform 448-wide
n-chunks; scores run in groups of THREE m-blocks into 3-bank PSUM slots
so one ACT instruction exps 3*448 elements (9 exp instructions per chunk
instead of 13); exps land in one contiguous per-chunk E buffer whose
pair-slices are exactly the DoubleRow rhs AP. Two of the eight 3-block
groups per chunk are exp'd on the DVE instead of ACT via a Schraudolph
bit-trick (clamp at the fp8-denormal floor, then bits = S*8/ln2 + const
converted to uint8 and bitcast as fp8e4) - this offloads ~22% of the
exp work from the ACT engine, which is otherwise the co-bottleneck.
Input DMA issues the three projection weights first on the scalar HWDGE
queue and the leading x chunks on sync, so the first matmuls start
~2.5us into the kernel instead of ~11us; chunk 0's score groups consume
x strictly in DMA arrival order, so the load pipeline stays ahead.
Output stores rotate across the sync/gpsimd/scalar queues, and the last
chunk's stores are split in quarters to shorten the final flight.
"""

import os
import sys

import numpy as np

for _p in (
    "/opt/trn_rl_repo",
    "/root/.axon_site",
    "/root/.axon_site/_ro/trn_rl_repo",
    "/root/.axon_site/_ro/pypackages",
):
    if _p not in sys.path and os.path.isdir(_p):
        sys.path.append(_p)

import concourse.bass as bass  # noqa: E402
import concourse.bacc as bacc  # noqa: E402
import concourse.tile as tile  # noqa: E402
from concourse import mybir  # noqa: E402
from concourse.masks import make_identity  # noqa: E402

B, C, H, W = 8, 256, 56, 56
N = H * W  # 3136
CH = C // 2  # 128
P = 128

CW = 448  # attention n-chunk width; 7 * 448 = 3136, uniform chunks
NCHUNK = 7
MB = 25  # m blocks: 24 x 128 + 1 x 64
MB_TAIL = N - 24 * P  # 64
NPAIR = 12  # DoubleRow m-block pairs (0,1)...(22,23); mb 24 is the tail
GRP = 3  # m-blocks per score/exp group
NGRP = 8  # full groups (blocks 0..23); group index 8 = the 64-row tail
PCH = 392  # x DMA / cast / projection chunk, 8 x 392 = 3136
EXP_BIAS = -3.5
# Schraudolph fp8e4 exp on DVE: bits = S * (8/ln2) + (56.5 + EXP_BIAS*8/ln2)
SCH_K = 8.0 / float(np.log(2.0))
SCH_B = 56.5 + EXP_BIAS * SCH_K
SCH_CLAMP = -SCH_B / SCH_K  # S below this maps to bits<0 -> clamp to 0
DVE_GROUPS = (3, 6)  # exp groups per chunk computed on DVE instead of ACT

F32 = mybir.dt.float32
BF16 = mybir.dt.bfloat16
F8 = mybir.dt.float8e4
U8 = mybir.dt.uint8

N_CORES = 8


def _kernel_body(tc):
    nc = tc.nc
    x_d = nc.dram_tensor("x", [C, N], F32, kind="ExternalInput").ap()
    wth_d = nc.dram_tensor("w_theta", [CH, C], F32, kind="ExternalInput").ap()
    wph_d = nc.dram_tensor("w_phi", [CH, C], F32, kind="ExternalInput").ap()
    wg_d = nc.dram_tensor("w_g", [CH, C], F32, kind="ExternalInput").ap()
    wz_d = nc.dram_tensor("w_z", [C, CH], F32, kind="ExternalInput").ap()
    out_d = nc.dram_tensor("out", [C, N], F32, kind="ExternalOutput").ap()

    from contextlib import ExitStack

    with ExitStack() as ctx:
        consts = ctx.enter_context(tc.tile_pool(name="consts", bufs=1))
        tmpp = ctx.enter_context(tc.tile_pool(name="tmpp", bufs=2))
        rp = ctx.enter_context(tc.tile_pool(name="rp", bufs=2))
        ynp = ctx.enter_context(tc.tile_pool(name="ynp", bufs=2))
        outp = ctx.enter_context(tc.tile_pool(name="outp", bufs=4))
        psum = ctx.enter_context(tc.tile_pool(name="psum", bufs=2, space="PSUM"))
        psum_y = ctx.enter_context(tc.tile_pool(name="psum_y", bufs=1, space="PSUM"))
        psum_d = ctx.enter_context(tc.tile_pool(name="psum_d", bufs=1, space="PSUM"))

        # ---- persistent SBUF tiles ----
        x_f32 = [consts.tile([P, N], F32, tag=f"x{h}", name=f"x{h}") for h in range(2)]
        x_bf = [
            consts.tile([P, N], BF16, tag=f"xb{h}", name=f"xb{h}") for h in range(2)
        ]
        theta = consts.tile([P, N], BF16, tag="theta", name="theta")
        phi = consts.tile([P, N], BF16, tag="phi", name="phi")
        gT = consts.tile([P, MB * P], F8, tag="gT", name="gT")  # [m_local, mb*128+c]
        ebuf = [
            consts.tile([P, MB * CW], F8, tag=f"eb{h}", name=f"eb{h}") for h in range(2)
        ]
        identity = consts.tile([P, P], BF16, tag="identity", name="identity")
        ones8 = consts.tile([P, 2, P], F8, tag="ones8", name="ones8")
        expb = consts.tile([P, 1], F32, tag="expb", name="expb")
        w_raw = {
            k: consts.tile([CH, C], F32, tag=f"wraw_{k}", name=f"wraw_{k}")
            for k in ("th", "ph", "g")
        }
        w_bf = {
            k: consts.tile([CH, C], BF16, tag=f"wbf_{k}", name=f"wbf_{k}")
            for k in ("th", "ph", "g")
        }
        wz_raw = [
            consts.tile([P, CH], F32, tag=f"wzraw{h}", name=f"wzraw{h}")
            for h in range(2)
        ]
        wz_bf = [
            consts.tile([P, CH], BF16, tag=f"wzbf{h}", name=f"wzbf{h}")
            for h in range(2)
        ]
        wT = {
            k: [
                consts.tile([P, P], BF16, tag=f"wT_{k}{j}", name=f"wT_{k}{j}")
                for j in range(2)
            ]
            for k in ("th", "ph", "g")
        }
        wzT = [
            consts.tile([P, P], BF16, tag=f"wzT{h}", name=f"wzT{h}") for h in range(2)
        ]

        # ---- input DMA. Weights first (small, on the scalar HWDGE queue) so
        # the weight transposes start ~1.5us in; x front chunks on sync; the
        # back x chunks alternate queues. 392-col granularity (200KB/transfer)
        # keeps completion signals frequent enough for chunk 0 to consume x
        # in arrival order without a monolithic wait.
        for k, d in (("th", wth_d), ("ph", wph_d), ("g", wg_d)):
            nc.scalar.dma_start(out=w_raw[k][:], in_=d[:, :])
        for ci in range(2):
            sl = slice(ci * PCH, (ci + 1) * PCH)
            for h in range(2):
                nc.sync.dma_start(out=x_f32[h][:, sl], in_=x_d[h * P : (h + 1) * P, sl])
        for h in range(2):
            nc.gpsimd.dma_start(out=wz_raw[h][:], in_=wz_d[h * P : (h + 1) * P, :])
        for ci in range(2, N // PCH):
            sl = slice(ci * PCH, (ci + 1) * PCH)
            nc.sync.dma_start(out=x_f32[0][:, sl], in_=x_d[0:P, sl])
            nc.scalar.dma_start(out=x_f32[1][:, sl], in_=x_d[P : 2 * P, sl])

        make_identity(nc, identity)
        nc.vector.memset(expb, EXP_BIAS)
        nc.vector.memset(ones8, 1.0)

        # ---- weight casts + PE transposes ----
        for k in ("th", "ph", "g"):
            nc.vector.tensor_copy(out=w_bf[k][:], in_=w_raw[k][:])
        for h in range(2):
            nc.vector.tensor_copy(out=wz_bf[h][:], in_=wz_raw[h][:])

        def pe_transpose(dst, src):
            ps = psum.tile([P, P], BF16, tag="s", name="s")
            nc.tensor.transpose(ps[:], src, identity[:])
            nc.vector.tensor_copy(out=dst, in_=ps[:])

        for k in ("th", "ph", "g"):
            for j in range(2):
                pe_transpose(wT[k][j][:], w_bf[k][:, j * P : (j + 1) * P])
        for h in range(2):
            pe_transpose(wzT[h][:], wz_bf[h][:])

        # ---- deferred x casts + projection emitters (interleaved into
        # chunk 0's group loop). Casts stay on DVE: the ACT queue must hold
        # only exps once the attention pipeline starts.
        xc_done = [0]

        def ensure_xcast(upto):
            upto = min(N // PCH, upto)
            while xc_done[0] < upto:
                j = xc_done[0]
                sl = slice(j * PCH, (j + 1) * PCH)
                for h in range(2):
                    nc.vector.tensor_copy(out=x_bf[h][:, sl], in_=x_f32[h][:, sl])
                xc_done[0] = j + 1

        def emit_proj_chunk(wkey, dst, j):
            ensure_xcast(j + 1)
            sl = slice(j * PCH, (j + 1) * PCH)
            ps = psum.tile([P, GRP * 512], F32, tag="s", name="s")
            nc.tensor.matmul(
                ps[:, :PCH], wT[wkey][0][:], x_bf[0][:, sl], start=True, stop=False
            )
            nc.tensor.matmul(
                ps[:, :PCH], wT[wkey][1][:], x_bf[1][:, sl], start=False, stop=True
            )
            nc.vector.tensor_copy(out=dst[:, sl], in_=ps[:, :PCH])

        def emit_gt_block(mb):
            mw = P if mb < MB - 1 else MB_TAIL
            ensure_xcast((mb * P + mw - 1) // PCH + 1)
            msl = slice(mb * P, mb * P + mw)
            ps = psum.tile([P, GRP * 512], F32, tag="s", name="s")
            nc.tensor.matmul(
                ps[:mw, :P], x_bf[0][:, msl], wT["g"][0][:], start=True, stop=False
            )
            nc.tensor.matmul(
                ps[:mw, :P], x_bf[1][:, msl], wT["g"][1][:], start=False, stop=True
            )
            nc.vector.tensor_copy(out=gT[:mw, mb * P : (mb + 1) * P], in_=ps[:mw, :P])

        done = {"th": 0, "ph": 0, "gT": 0}  # chunks/blocks emitted so far

        def ensure(kind, upto):
            """Emit projection work up to (exclusive) index `upto`."""
            while done[kind] < upto:
                j = done[kind]
                if kind == "th":
                    emit_proj_chunk("th", theta, j)
                elif kind == "ph":
                    emit_proj_chunk("ph", phi, j)
                else:
                    emit_gt_block(j)
                done[kind] = j + 1

        NP_CH = N // PCH  # 8

        def ph_chunks_for_blocks(nblocks):
            return min(NP_CH, (nblocks * P + PCH - 1) // PCH)

        # upfront: enough for groups 0-1 of chunk 0 and the first PV pair
        ensure("th", 2)  # theta cols 0:784 >= 448
        ensure("ph", ph_chunks_for_blocks(6))  # phi blocks 0..5
        ensure("gT", 2)  # m-blocks 0,1

        # ---- attention ----
        DR = mybir.MatmulPerfMode.DoubleRow

        def emit_group(ci, cs, ebc, interleave, g):
            """Scores + exp for score-group g (3 m-blocks, or the 64-row tail
            when g == NGRP) of this chunk, into the chunk's E buffer."""
            if interleave:
                if g < NGRP:
                    ensure("ph", ph_chunks_for_blocks(3 * g + 6))
                else:
                    ensure("ph", NP_CH)
                ensure("gT", min(MB, 3 * g + 4))
                ensure("th", min(NP_CH, g + 3))
            if g < NGRP:
                s_ps = psum.tile([P, GRP * 512], F32, tag="s", name="s")
                for j in range(GRP):
                    mb = GRP * g + j
                    nc.tensor.matmul(
                        s_ps[:, j * 512 : j * 512 + CW],
                        phi[:, mb * P : (mb + 1) * P],
                        theta[:, cs : cs + CW],
                        start=True,
                        stop=True,
                    )
                sview = s_ps[:].rearrange("p (a b) -> p a b", a=GRP)[:, :, :CW]
                dst = ebuf[ebc][:, GRP * g * CW : (GRP * g + GRP) * CW]
                if g in DVE_GROUPS:
                    # Schraudolph exp: clamp, then bits = S*K + B as uint8,
                    # read back as fp8e4. ~6% elementwise error, same order
                    # as fp8 rounding; output error is damped ~70x by the
                    # residual connection.
                    t_t = tmpp.tile([P, GRP * CW], F32, tag="t", name="t")
                    nc.vector.tensor_scalar(
                        out=t_t[:].rearrange("p (a b) -> p a b", a=GRP),
                        in0=sview,
                        scalar1=float(SCH_CLAMP),
                        scalar2=None,
                        op0=mybir.AluOpType.max,
                    )
                    nc.vector.tensor_scalar(
                        out=dst.bitcast(U8),
                        in0=t_t[:],
                        scalar1=float(SCH_K),
                        scalar2=float(SCH_B),
                        op0=mybir.AluOpType.mult,
                        op1=mybir.AluOpType.add,
                    )
                else:
                    nc.scalar.activation(
                        out=dst.rearrange("p (a b) -> p a b", a=GRP),
                        in_=sview,
                        func=mybir.ActivationFunctionType.Exp,
                        bias=expb[:],
                    )
            else:
                mb = MB - 1
                s_ps = psum.tile([P, GRP * 512], F32, tag="s", name="s")
                nc.tensor.matmul(
                    s_ps[:MB_TAIL, :CW],
                    phi[:, mb * P : mb * P + MB_TAIL],
                    theta[:, cs : cs + CW],
                    start=True,
                    stop=True,
                )
                nc.scalar.activation(
                    out=ebuf[ebc][:MB_TAIL, mb * CW : (mb + 1) * CW],
                    in_=s_ps[:MB_TAIL, :CW],
                    func=mybir.ActivationFunctionType.Exp,
                    bias=expb[:MB_TAIL],
                )

        def emit_unit(ebc, y_ps, d_ps, u):
            """PV + denominator matmuls for m-unit u (a DoubleRow pair, or the
            64-row tail when u == NPAIR)."""
            first, last = u == 0, u == NPAIR
            eb = ebuf[ebc]
            if u < NPAIR:
                gpair = gT[:, 2 * u * P : (2 * u + 2) * P].rearrange(
                    "p (k c) -> p k c", k=2
                )
                et = eb[:, 2 * u * CW : (2 * u + 2) * CW].rearrange(
                    "p (k c) -> p k c", k=2
                )
                nc.tensor.matmul(
                    y_ps[:, :CW], gpair, et, start=first, stop=last, perf_mode=DR
                )
                nc.tensor.matmul(
                    d_ps[:, :CW], ones8[:], et, start=first, stop=last, perf_mode=DR
                )
            else:
                mb = MB - 1
                et = eb[:MB_TAIL, mb * CW : (mb + 1) * CW]
                nc.tensor.matmul(
                    y_ps[:, :CW],
                    gT[:MB_TAIL, mb * P : (mb + 1) * P],
                    et,
                    start=first,
                    stop=last,
                )
                nc.tensor.matmul(
                    d_ps[:, :CW],
                    ones8[:MB_TAIL, 0, :],
                    et,
                    start=first,
                    stop=last,
                )

        out_queues = [nc.sync, nc.gpsimd, nc.scalar]
        _q_rr = [0]

        def emit_store(h, cs, o_t, nsplit):
            half = (CW + nsplit - 1) // nsplit
            for s in range(nsplit):
                so = s * half
                sw = min(half, CW - so)
                if sw <= 0:
                    continue
                q = out_queues[_q_rr[0] % 3]
                _q_rr[0] += 1
                q.dma_start(
                    out=out_d[h * P : (h + 1) * P, cs + so : cs + so + sw],
                    in_=o_t[:, so : so + sw],
                )

        def make_epilogue(ci, cs, y_ps, d_ps):
            def _ep():
                r_t = rp.tile([P, CW], F32, tag="r", name="r")
                nc.vector.reciprocal_approx_fast(out=r_t[:, :CW], in_=d_ps[:, :CW])
                yn_t = ynp.tile([P, CW], BF16, tag="yn", name="yn")
                nc.vector.tensor_mul(
                    out=yn_t[:, :CW], in0=y_ps[:, :CW], in1=r_t[:, :CW]
                )
                # z = w_z @ ynorm + x -> out. h=0 reuses the d bank (freed
                # after the recip), h=1 the y bank.
                for h, ztag in ((0, "d"), (1, "y")):
                    zpool = psum_y if ztag == "y" else psum_d
                    z_ps = zpool.tile([P, 512], F32, tag=ztag, name="zps")
                    nc.tensor.matmul(
                        z_ps[:, :CW],
                        wzT[h][:],
                        yn_t[:, :CW],
                        start=True,
                        stop=True,
                    )
                    o_t = outp.tile([P, CW], F32, tag="o", name="o")
                    nc.vector.tensor_add(
                        out=o_t[:, :CW], in0=z_ps[:, :CW], in1=x_f32[h][:, cs : cs + CW]
                    )
                    emit_store(h, cs, o_t, nsplit=2 if ci < NCHUNK - 1 else 4)

            return _ep

        pending = [None]

        def flush():
            if pending[0] is not None:
                pending[0]()
                pending[0] = None

        def chunk(ci):
            cs = ci * CW
            ebc = ci % 2
            interleave = ci == 0
            y_ps = psum_y.tile([P, 512], F32, tag="y", name="y")
            d_ps = psum_d.tile([P, 512], F32, tag="d", name="d")
            emit_group(ci, cs, ebc, interleave, 0)
            emit_group(ci, cs, ebc, interleave, 1)
            flush()  # previous chunk's epilogue overlaps this chunk's scores
            next_u = [0]

            def drain_units(upto):
                while next_u[0] < upto:
                    emit_unit(ebc, y_ps, d_ps, next_u[0])
                    next_u[0] += 1

            for g in range(2, NGRP + 1):
                emit_group(ci, cs, ebc, interleave, g)
                # PV units whose exp instructions are already queued a group
                # back: after group g is emitted, blocks 0..3g-1 are in
                # flight; stay one group behind so the PE never waits.
                drain_units(min(NPAIR, (3 * g - 1) // 2))
            drain_units(NPAIR + 1)
            pending[0] = make_epilogue(ci, cs, y_ps, d_ps)

        for ci in range(NCHUNK):
            chunk(ci)
        flush()

        assert done == {"th": NP_CH, "ph": NP_CH, "gT": MB}, done


_NC_CACHE = None


def build_nc():
    global _NC_CACHE
    if _NC_CACHE is None:
        nc = bacc.Bacc("TRN2", target_bir_lowering=False, debug=False)
        with tile.TileContext(nc) as tc:
            _kernel_body(tc)
        nc.compile()
        _NC_CACHE = nc
    return _NC_CACHE


def kernel(x, w_theta, w_phi, w_g, w_z, trace=False):
    assert x.shape == (B, C, H, W), x.shape
    nc = build_nc()
    from concourse.bass_utils import run_bass_kernel_spmd

    shared = {
        "w_theta": np.ascontiguousarray(w_theta, dtype=np.float32),
        "w_phi": np.ascontiguousarray(w_phi, dtype=np.float32),
        "w_g": np.ascontiguousarray(w_g, dtype=np.float32),
        "w_z": np.ascontiguousarray(w_z, dtype=np.float32),
    }
    in_maps = [
        dict(shared, x=np.ascontiguousarray(x[i].reshape(C, N), dtype=np.float32))
        for i in range(N_CORES)
    ]
    res = run_bass_kernel_spmd(
        nc, in_maps, core_ids=list(range(N_CORES)), trace=trace
    )
    out = np.stack([res.results[i]["out"].reshape(C, H, W) for i in range(N_CORES)])
    kernel.last_result = res
    return out


kernel.last_result = None
